# revision 44
# baseline (speedup 1.0000x reference)
"""Trainium2 Bass kernel for nn_ConcatHandshaking.

Computes out[b, p, :] = tanh(proj_i[b, ii[p], :] + proj_j[b, jj[p], :])
where proj_i = hidden @ W[:D], proj_j = hidden @ W[D:] + bias, and (ii, jj)
are the upper-triangular token pairs of a length-S sequence.

Sharding: data-parallel over batch. B=16 batches -> 2 per core on 8 cores.

All on-device data is fp16 (host casts inputs, output cast back to f32 on
host; fp16 keeps max rel err ~2.6e-3 vs the 2e-2 gate). This halves the
dominant HBM traffic (the 8256x768 output per batch) versus f32.

The kernel is ACT-bound: tanh only runs on ScalarE at 1 elem/cycle/lane,
so the 2*8256*768 = 12.68M output elems per core cost >=82.5us of ACT
busy (plus ~260ns/instr overhead, 66 instrs -> ~97us). Everything else
(PE ~108us busy incl. LDWEIGHTS, stores ~71us at 358GB/s) must hide
under the ACT stream, so the schedule optimizes (a) time-to-first-
ACTIVATE and (b) the post-ACT store drain. There is also an ~8us fixed
NEFF preamble (engine init + instruction fetch) before any DMA flows.

Per-core pipeline:
  Stage A: hidden arrives host-packed as [128, BPC, D/128, S] (3KB DMA
           lines, one load); W host-packed per k-chunk as [D/128, 128, 2, H]
           so each chunk (both halves) is one 3KB-line load, issued in
           need-order alternating across the two HWDGE rings. fp16
           matmuls hidT.T x W-half accumulate both batches' projections
           in PSUM f32 chunk-by-chunk as the W chunks land; bias folds in
           as a K=1 ones-vector matmul; proj_i drains on DVE, proj_j on
           ScalarE (both idle before the tanh stream starts).
  Stage B: pair axis (P=8256) split into 64 full tiles of 128 pairs plus a
           64-pair tail. Tile t (t<64) holds pairs 1024*(t//8) + 8*m + (t%8),
           m=0..127, so each PSUM partition m of an 8-tile output group
           holds 8 CONSECUTIVE out rows -> 12KB contiguous DMA
           descriptors (the store ring must sustain the ACT production
           rate of ~248 GB/s or the drain tail grows). Each tile is
           processed for BOTH batches under one selector weight load
           (stationary reuse: 1 LDWEIGHTS per selector per tile instead
           of per batch), accumulating
           selI.T @ proj_i + selJ.T @ proj_j into one 4-bank PSUM tile
           (batch0 at cols 0:768, batch1 at 1024:1792). One ScalarE tanh
           drains both batches' rows into an fp16 staging group; groups
           0..5 are stored with ~0.79MB per-batch DMAs, groups 6..7 in
           2-tile (0.39MB) pieces as activations land, so the
           post-compute drain is short.

Selectors are fp8 (0/1 exact) in DRAM, CHUNK-MAJOR [4, S, 16, S]: a
16-tile chunk load has 2KB-contiguous per-partition lines ([S, NT, S]
made the DMA read 128-byte DRAM fragments at ~70 GB/s and gated the
ramp; dma_start_transpose of an fp16 pair-major layout was even worse,
16640 256-byte xbar descriptors clogging the ring to 39us). The two
tail-tile selectors ride one [S, 2, S] tensor. Casts to fp16 run on
GPSIMD (otherwise idle; ~3.4 cyc/elem is plenty ahead of use), with
chunk 0 split per-GROUP so tile 0's weights are ready early. The PE
must NOT consume fp8 selectors directly: a mixed fp8-lhsT x fp16-rhs
matmul runs at HALF rate on HW (450ns vs 216ns per 512 cols). The DVE
runs ONLY the two proj_i PSUM->SBUF copies: the Tile scheduler
dispatches by operand readiness, not program order, so anything else
queued there (e.g. casts) jumps ahead of the copies and stalls tile 0.
All stores go on the sync (SP) HWDGE ring; every scalar-ring
(ACT-issued) dma_start is emitted before the first ACTIVATE so the ACT
queue never stalls on DMA issue mid-stream.
"""

import sys

if "/opt/trn_rl_repo" not in sys.path:
    sys.path.insert(0, "/opt/trn_rl_repo")

from contextlib import ExitStack

import numpy as np

import concourse.bass as bass
import concourse.mybir as mybir
import concourse.tile as tile
from concourse import bacc
from concourse.bass_utils import run_bass_kernel_spmd

B, S, D, H = 16, 128, 768, 768
P = S * (S + 1) // 2  # 8256 upper-triangular pairs
NCORES = 8
BPC = B // NCORES  # batches per core
NFT = 64  # full pair tiles of 128 pairs (covering 8192 pairs)
NT = NFT + 1  # plus one 64-pair tail tile
TAIL = P - NFT * S  # 64
GRP = 8  # pair tiles per output staging group (1024 pairs, ~1.57MB fp16)
NGRP = NFT // GRP  # 8 full groups per batch

F32 = mybir.dt.float32
F16 = mybir.dt.float16
FP8 = mybir.dt.float8e4

TANH = mybir.ActivationFunctionType.Tanh

SEL_CH = 8  # selector tiles per DRAM chunk (1KB-contiguous lines)
NSC = NFT // SEL_CH  # 8 full-tile chunks; the tail tile rides sel_t


def _build_nc(repeat: int = 1) -> bass.Bass:
    nc = bacc.Bacc(
        "TRN2", target_bir_lowering=False, debug=False, num_devices=NCORES
    )

    NCH = D // 128  # 6 k-chunks

    # hid host-packed [128(k), BPC, NCH, S]: one load, 3KB lines.
    hidt = nc.declare_dram_parameter("hidt", [128, BPC, NCH, S], F16, isOutput=False)
    # W host-packed [2, 128(k), NCH/2, 2(half), H]: two 1.18MB loads with
    # 9KB-contiguous lines (3KB-line per-chunk loads only reached ~130
    # GB/s against the early-DMA latency floor).
    w = nc.declare_dram_parameter("w", [2, 128, NCH // 2, 2, H], F16, isOutput=False)
    bias = nc.declare_dram_parameter("bias", [1, H], F16, isOutput=False)
    # selectors chunk-major for contiguous direct loads.
    sel_i = nc.declare_dram_parameter("sel_i", [NSC, S, SEL_CH, S], FP8, isOutput=False)
    sel_j = nc.declare_dram_parameter("sel_j", [NSC, S, SEL_CH, S], FP8, isOutput=False)
    sel_t = nc.declare_dram_parameter("sel_t", [S, 2, S], FP8, isOutput=False)
    out = nc.declare_dram_parameter("out", [BPC, P, H], F16, isOutput=True)

    with tile.TileContext(nc) as tc, ExitStack() as ctx:
        consts = ctx.enter_context(tc.tile_pool(name="consts", bufs=1))
        acts = ctx.enter_context(tc.tile_pool(name="acts", bufs=2))
        outs = ctx.enter_context(tc.tile_pool(name="outs", bufs=2))
        # One shared PSUM pool: every tile is [128, 2048] f32 = 4 banks;
        # bufs=2 -> all 8 banks. start=True clears a whole PSUM bank, so the
        # two 768-wide sub-tiles must not share a bank: offsets 0 and 1024.
        psum = ctx.enter_context(tc.tile_pool(name="psum", bufs=2, space="PSUM"))

        # ---- constants computed on-engine FIRST: the PE warmup matmuls
        # depend on warm_w, and anything queued ahead of these memsets on
        # the GpSimd queue would stall the whole PE stream.
        ones_sb = consts.tile([1, 128], F16, name="ones")
        nc.gpsimd.memset(ones_sb[:], 1.0)
        warm_w = consts.tile([128, 128], F16, name="warm_w")
        nc.gpsimd.memset(warm_w[:], 0.125)

        # ---- SBUF tiles
        selib_i = consts.tile([S, NFT, S], FP8, name="selib_i")
        selib_j = consts.tile([S, NFT, S], FP8, name="selib_j")
        selib_t = consts.tile([S, 2, S], FP8, name="selib_t")
        seli_mm = consts.tile([S, NT, S], F16, name="seli_mm")
        selj_mm = consts.tile([S, NT, S], F16, name="selj_mm")
        w_sb = consts.tile([128, NCH, 2, H], F16, name="w_sb")
        bias_sb = consts.tile([1, H], F16, name="bias_sb")
        hid_sb = consts.tile([128, BPC, NCH, S], F16, name="hid_sb")

        # ---- loads, need-ordered across the two HWDGE rings. The early
        # loads share the ~358 GB/s per-core HBM cap (the sync ring
        # sustains ~205 GB/s, the scalar ring only ~150-170, so the
        # scalar ring carries fewer bytes). Combined need order: hid+Wc0
        # gate the first stage-A matmul; selector group 0 (0.26MB) rides
        # early for its DVE cast; remaining W chunks in consumption
        # order; then the bulk selectors. All scalar-ring (ACT-issued)
        # dma_starts complete their issue long before the first ACTIVATE.
        nc.sync.dma_start(w_sb[:, 0:3, :, :], w[0])
        nc.scalar.dma_start(hid_sb[:], hidt[:])
        nc.sync.dma_start(w_sb[:, 3:6, :, :], w[1])
        nc.scalar.dma_start(selib_i[:, 0:SEL_CH, :], sel_i[0])
        nc.scalar.dma_start(selib_j[:, 0:SEL_CH, :], sel_j[0])
        nc.sync.dma_start(bias_sb[:], bias[:])
        # bulk selectors: 8-tile chunks on the scalar ring (the sync ring
        # carries all output stores).
        for sc in range(1, NSC):
            nc.scalar.dma_start(
                selib_i[:, sc * SEL_CH : (sc + 1) * SEL_CH, :], sel_i[sc]
            )
            nc.scalar.dma_start(
                selib_j[:, sc * SEL_CH : (sc + 1) * SEL_CH, :], sel_j[sc]
            )
        nc.scalar.dma_start(selib_t[:], sel_t[:])

        # group-0 selector casts on the DVE (1.2us each, finished well
        # before the stage-A copies are ready, so no FIFO collision); all
        # later groups + the tail on GPSIMD (~3.6us each, serial, but the
        # deadline for group g is ~1.5us * 8g after the stream starts).
        nc.vector.tensor_copy(seli_mm[:, 0:SEL_CH, :], selib_i[:, 0:SEL_CH, :])
        nc.vector.tensor_copy(selj_mm[:, 0:SEL_CH, :], selib_j[:, 0:SEL_CH, :])
        for sc in range(1, NSC):
            sl = slice(sc * SEL_CH, (sc + 1) * SEL_CH)
            nc.gpsimd.tensor_copy(seli_mm[:, sl, :], selib_i[:, sl, :])
            nc.gpsimd.tensor_copy(selj_mm[:, sl, :], selib_j[:, sl, :])
            if sc == 3:
                # tail selector cast before chunk 4: the tail tile is
                # processed mid-stream at group 3 (~tile 28's slot).
                nc.gpsimd.tensor_copy(
                    seli_mm[:, NFT : NFT + 1, :], selib_t[:, 0:1, :]
                )
                nc.gpsimd.tensor_copy(
                    selj_mm[:, NFT : NFT + 1, :], selib_t[:, 1:2, :]
                )

        # PE warmup: ~1.5us of junk matmuls so the HAM clock-gate opens
        # before the real matmuls arrive (cold PE runs at half rate).
        warm_ps = psum.tile([128, 2048], F32, name="warm_ps", tag="ps")
        for k in range(28):
            nc.tensor.matmul(
                warm_ps[:, 0:128], lhsT=warm_w[:], rhs=warm_w[:], start=True, stop=True
            )

        for rep in range(repeat):
            # ---- stage A: projections for both batches --------------------
            # Both batches' PSUM tiles live at once (8 banks) and the chunk
            # matmuls interleave across batches, so the accumulation chases
            # the per-chunk W loads down the rings.
            pps = [
                psum.tile([128, 2048], F32, name=f"pp_{rep}_{bb}", tag="ps")
                for bb in range(BPC)
            ]
            for c in range(NCH - 1):
                first = c == 0
                for bb in range(BPC):
                    pp = pps[bb]
                    hT = hid_sb[:, bb, c, :]
                    nc.tensor.matmul(
                        pp[:, 0:512], lhsT=hT, rhs=w_sb[:, c, 0, 0:512],
                        start=first, stop=False,
                    )
                    nc.tensor.matmul(
                        pp[:, 512:768], lhsT=hT, rhs=w_sb[:, c, 0, 512:768],
                        start=first, stop=False,
                    )
                    nc.tensor.matmul(
                        pp[:, 1024:1536], lhsT=hT, rhs=w_sb[:, c, 1, 0:512],
                        start=first, stop=False,
                    )
                    nc.tensor.matmul(
                        pp[:, 1536:1792], lhsT=hT, rhs=w_sb[:, c, 1, 512:768],
                        start=first, stop=False,
                    )
            # last chunk + bias + copies PER BATCH, so batch 0's
            # projections finish (and tile 0's matmul ladder can start)
            # while batch 1's last chunk is still on the PE.
            c = NCH - 1
            pis, pjs = [], []
            for bb in range(BPC):
                pp = pps[bb]
                hT = hid_sb[:, bb, c, :]
                nc.tensor.matmul(
                    pp[:, 0:512], lhsT=hT, rhs=w_sb[:, c, 0, 0:512],
                    start=False, stop=True,
                )
                nc.tensor.matmul(
                    pp[:, 512:768], lhsT=hT, rhs=w_sb[:, c, 0, 512:768],
                    start=False, stop=True,
                )
                nc.tensor.matmul(
                    pp[:, 1024:1536], lhsT=hT, rhs=w_sb[:, c, 1, 0:512],
                    start=False, stop=False,
                )
                nc.tensor.matmul(
                    pp[:, 1536:1792], lhsT=hT, rhs=w_sb[:, c, 1, 512:768],
                    start=False, stop=False,
                )
                # bias folded in as a K=1 matmul of ones.T @ b
                nc.tensor.matmul(
                    pp[:, 1024:1536], lhsT=ones_sb[:], rhs=bias_sb[:, 0:512],
                    start=False, stop=True,
                )
                nc.tensor.matmul(
                    pp[:, 1536:1792], lhsT=ones_sb[:], rhs=bias_sb[:, 512:768],
                    start=False, stop=True,
                )
                pi = acts.tile([S, H], F16, name=f"pi_{rep}_{bb}")
                pj = acts.tile([S, H], F16, name=f"pj_{rep}_{bb}")
                # proj_i on DVE, proj_j on ScalarE: both are idle during the
                # ramp, so the four PSUM->SBUF copies drain two-at-a-time.
                nc.vector.tensor_copy(pi[:], pp[:, 0:768])
                nc.scalar.copy(pj[:], pp[:, 1024:1792])
                pis.append(pi)
                pjs.append(pj)

            # ---- stage B: pair tiles, both batches per selector load ------
            for g in range(NGRP):
                t0 = g * GRP
                og2 = outs.tile(
                    [128, BPC, GRP, H], F16, name=f"og_{rep}_{g}", tag="og"
                )
                for tt in range(GRP):
                    t = t0 + tt
                    pq = psum.tile(
                        [128, 2048], F32, name=f"pq_{rep}_{t}", tag="ps"
                    )
                    for sel, prs, st in (
                        (seli_mm, pis, True),
                        (selj_mm, pjs, False),
                    ):
                        nc.tensor.matmul(
                            pq[:, 0:512], lhsT=sel[:, t, :],
                            rhs=prs[0][:, 0:512], start=st, stop=not st,
                        )
                        nc.tensor.matmul(
                            pq[:, 512:768], lhsT=sel[:, t, :],
                            rhs=prs[0][:, 512:768], start=st, stop=not st,
                        )
                        nc.tensor.matmul(
                            pq[:, 1024:1536], lhsT=sel[:, t, :],
                            rhs=prs[1][:, 0:512], start=st, stop=not st,
                        )
                        nc.tensor.matmul(
                            pq[:, 1536:1792], lhsT=sel[:, t, :],
                            rhs=prs[1][:, 512:768], start=st, stop=not st,
                        )
                    nc.scalar.activation(
                        og2[:, :, tt, :],
                        pq.rearrange("p (t x) -> p t x", t=2)[:, :, 0:768],
                        TANH,
                    )
                    r0 = t0 * S
                    dsts = [
                        out[bb, r0 : r0 + GRP * S, :].rearrange(
                            "(p g) h -> p g h", p=128
                        )
                        for bb in range(BPC)
                    ]
                    if g == NGRP - 2 and tt in (1, 3, 5, 7):
                        # group 6: 2-tile (0.39MB) pieces as activations land
                        for bb in range(BPC):
                            nc.sync.dma_start(
                                dsts[bb][:, tt - 1 : tt + 1, :],
                                og2[:, bb, tt - 1 : tt + 1, :],
                            )
                    elif g == NGRP - 1 and (tt in (1, 3, 5) or tt >= 6):
                        # group 7: 2-tile pieces, then per-tile at the very
                        # end so the post-compute drain is one tile deep.
                        sl = slice(tt - 1, tt + 1) if tt < 6 else slice(tt, tt + 1)
                        for bb in range(BPC):
                            nc.sync.dma_start(
                                dsts[bb][:, sl, :], og2[:, bb, sl, :]
                            )
                if g < NGRP - 2:
                    for bb in range(BPC):
                        nc.sync.dma_start(dsts[bb], og2[:, bb, :, :])
                if g == 3:
                    # ---- tail: 64 pairs via selector tile NFT, processed
                    # mid-stream (after the bulk selector load has landed)
                    # so its ACT/store latency hides ----------------------
                    pqt = psum.tile([128, 2048], F32, name=f"pqt_{rep}", tag="ps")
                    for sel, prs, st in (
                        (seli_mm, pis, True),
                        (selj_mm, pjs, False),
                    ):
                        nc.tensor.matmul(
                            pqt[:, 0:512], lhsT=sel[:, NFT, :],
                            rhs=prs[0][:, 0:512], start=st, stop=not st,
                        )
                        nc.tensor.matmul(
                            pqt[:, 512:768], lhsT=sel[:, NFT, :],
                            rhs=prs[0][:, 512:768], start=st, stop=not st,
                        )
                        nc.tensor.matmul(
                            pqt[:, 1024:1536], lhsT=sel[:, NFT, :],
                            rhs=prs[1][:, 0:512], start=st, stop=not st,
                        )
                        nc.tensor.matmul(
                            pqt[:, 1536:1792], lhsT=sel[:, NFT, :],
                            rhs=prs[1][:, 512:768], start=st, stop=not st,
                        )
                    og2t = acts.tile([128, BPC, H], F16, name=f"ogt_{rep}")
                    nc.scalar.activation(
                        og2t[0:TAIL, :, :],
                        pqt.rearrange("p (t x) -> p t x", t=2)[0:TAIL, :, 0:768],
                        TANH,
                    )
                    for bb in range(BPC):
                        nc.sync.dma_start(
                            out[bb, NFT * S : P, :], og2t[0:TAIL, bb, :]
                        )

    nc.compile()
    return nc


_NC_CACHE: dict[int, bass.Bass] = {}
LAST_RESULTS = None  # BassKernelResults of the most recent kernel() call


def _get_nc(repeat: int = 1) -> bass.Bass:
    if repeat not in _NC_CACHE:
        _NC_CACHE[repeat] = _build_nc(repeat)
    return _NC_CACHE[repeat]


_SEL_CACHE = None


def _selectors() -> tuple[np.ndarray, np.ndarray, np.ndarray]:
    """0/1 selector matrices, fp8 (exact), chunk-major [NSC, S, SEL_CH, S]
    (chunk, token k, tile-within-chunk, column m) so each chunk load has
    2KB-contiguous per-partition lines. Tile t<64 column m selects pair
    1024*(t//8) + 8*m + (t%8); the tail selectors (pair 8192+m in column
    m<64) ride sel_t[S, 2, S] = (k, {i,j}, m)."""
    global _SEL_CACHE
    if _SEL_CACHE is not None:
        return _SEL_CACHE
    import ml_dtypes

    ii, jj = np.triu_indices(S)
    sel_i = np.zeros((NSC, S, SEL_CH, S), dtype=np.float32)
    sel_j = np.zeros((NSC, S, SEL_CH, S), dtype=np.float32)
    m = np.arange(S)
    for t in range(NFT):
        pr = 1024 * (t // 8) + 8 * m + (t % 8)
        sel_i[t // SEL_CH, ii[pr], t % SEL_CH, m] = 1.0
        sel_j[t // SEL_CH, jj[pr], t % SEL_CH, m] = 1.0
    sel_t = np.zeros((S, 2, S), dtype=np.float32)
    mt = np.arange(TAIL)
    pr = NFT * S + mt
    sel_t[ii[pr], 0, mt] = 1.0
    sel_t[jj[pr], 1, mt] = 1.0
    _SEL_CACHE = (
        sel_i.astype(ml_dtypes.float8_e4m3),
        sel_j.astype(ml_dtypes.float8_e4m3),
        sel_t.astype(ml_dtypes.float8_e4m3),
    )
    return _SEL_CACHE


def kernel(hidden: np.ndarray, W: np.ndarray, b: np.ndarray) -> np.ndarray:
    hidden = np.asarray(hidden, dtype=np.float32)
    W = np.asarray(W, dtype=np.float32)
    b = np.asarray(b, dtype=np.float32)

    sel_i, sel_j, sel_t = _selectors()
    # hidden packed per core to [128(k), BPC, NCH, S]: per-partition lines
    # are BPC*NCH*S*2 = 3KB contiguous, loaded in one dma_start.
    # hidden[b, s, d] with d = c*128 + k  ->  hidt[k, b, c, s]
    hidt = np.ascontiguousarray(
        hidden.transpose(2, 0, 1)  # [D, B, S]
        .reshape(D // 128, 128, B, S)
        .transpose(1, 2, 0, 3)  # [128(k), B, NCH, S]
        .astype(np.float16)
    )
    # W packed to [2, 128(k), NCH/2, 2(half), H]: two loads with
    # 9KB-contiguous per-partition lines.
    # W[d, h] with d = half*768 + c*128 + k -> w[c//3, k, c%3, half, h]
    w16 = np.ascontiguousarray(
        W.reshape(2, 2, 3, 128, H)  # [half, cgrp, c%3, k, h]
        .transpose(1, 3, 2, 0, 4)  # [cgrp, k, c%3, half, h]
        .astype(np.float16)
    )
    b16 = b.astype(np.float16).reshape(1, H)

    nc = _get_nc()
    in_maps = []
    for c in range(NCORES):
        in_maps.append(
            {
                "hidt": np.ascontiguousarray(hidt[:, c * BPC : (c + 1) * BPC]),
                "w": w16,
                "bias": b16,
                "sel_i": sel_i,
                "sel_j": sel_j,
                "sel_t": sel_t,
            }
        )
    res = run_bass_kernel_spmd(nc, in_maps, list(range(NCORES)))
    global LAST_RESULTS
    LAST_RESULTS = res
    out = np.concatenate([res.results[c]["out"] for c in range(NCORES)], axis=0)
    return out.astype(np.float32)


# revision 48
# speedup vs baseline: 1.0048x; 1.0048x over previous
"""Trainium2 Bass kernel for nn_ConcatHandshaking.

Computes out[b, p, :] = tanh(proj_i[b, ii[p], :] + proj_j[b, jj[p], :])
where proj_i = hidden @ W[:D], proj_j = hidden @ W[D:] + bias, and (ii, jj)
are the upper-triangular token pairs of a length-S sequence.

Sharding: data-parallel over batch. B=16 batches -> 2 per core on 8 cores.

All on-device data is fp16 (host casts inputs, output cast back to f32 on
host; fp16 keeps max rel err ~2.6e-3 vs the 2e-2 gate). This halves the
dominant HBM traffic (the 8256x768 output per batch) versus f32.

The kernel is ACT-bound: tanh only runs on ScalarE at 1 elem/cycle/lane,
so the 2*8256*768 = 12.68M output elems per core cost >=82.5us of ACT
busy (plus ~260ns/instr overhead, 66 instrs -> ~97us). Everything else
(PE ~108us busy incl. LDWEIGHTS, stores ~71us at 358GB/s) must hide
under the ACT stream, so the schedule optimizes (a) time-to-first-
ACTIVATE and (b) the post-ACT store drain. There is also an ~8us fixed
NEFF preamble (engine init + instruction fetch) before any DMA flows.

Per-core pipeline:
  Stage A: hidden arrives host-packed as [128, BPC, D/128, S] (3KB DMA
           lines, one load); W host-packed per k-chunk as [D/128, 128, 2, H]
           so each chunk (both halves) is one 3KB-line load, issued in
           need-order alternating across the two HWDGE rings. fp16
           matmuls hidT.T x W-half accumulate both batches' projections
           in PSUM f32 chunk-by-chunk as the W chunks land; bias folds in
           as a K=1 ones-vector matmul; proj_i drains on DVE, proj_j on
           ScalarE (both idle before the tanh stream starts).
  Stage B: pair axis (P=8256) split into 64 full tiles of 128 pairs plus a
           64-pair tail. Tile t (t<64) holds pairs 1024*(t//8) + 8*m + (t%8),
           m=0..127, so each PSUM partition m of an 8-tile output group
           holds 8 CONSECUTIVE out rows -> 12KB contiguous DMA
           descriptors (the store ring must sustain the ACT production
           rate of ~248 GB/s or the drain tail grows). Each tile is
           processed for BOTH batches under one selector weight load
           (stationary reuse: 1 LDWEIGHTS per selector per tile instead
           of per batch), accumulating
           selI.T @ proj_i + selJ.T @ proj_j into one 4-bank PSUM tile
           (batch0 at cols 0:768, batch1 at 1024:1792). One ScalarE tanh
           drains both batches' rows into an fp16 staging group; groups
           0..5 are stored with ~0.79MB per-batch DMAs, groups 6..7 in
           2-tile (0.39MB) pieces as activations land, so the
           post-compute drain is short.

Selectors are fp8 (0/1 exact) in DRAM, CHUNK-MAJOR [4, S, 16, S]: a
16-tile chunk load has 2KB-contiguous per-partition lines ([S, NT, S]
made the DMA read 128-byte DRAM fragments at ~70 GB/s and gated the
ramp; dma_start_transpose of an fp16 pair-major layout was even worse,
16640 256-byte xbar descriptors clogging the ring to 39us). The two
tail-tile selectors ride one [S, 2, S] tensor. Casts to fp16 run on
GPSIMD (otherwise idle; ~3.4 cyc/elem is plenty ahead of use), with
chunk 0 split per-GROUP so tile 0's weights are ready early. The PE
must NOT consume fp8 selectors directly: a mixed fp8-lhsT x fp16-rhs
matmul runs at HALF rate on HW (450ns vs 216ns per 512 cols). The DVE
runs ONLY the two proj_i PSUM->SBUF copies: the Tile scheduler
dispatches by operand readiness, not program order, so anything else
queued there (e.g. casts) jumps ahead of the copies and stalls tile 0.
All stores go on the sync (SP) HWDGE ring; every scalar-ring
(ACT-issued) dma_start is emitted before the first ACTIVATE so the ACT
queue never stalls on DMA issue mid-stream.
"""

import sys

if "/opt/trn_rl_repo" not in sys.path:
    sys.path.insert(0, "/opt/trn_rl_repo")

from contextlib import ExitStack

import numpy as np

import concourse.bass as bass
import concourse.mybir as mybir
import concourse.tile as tile
from concourse import bacc
from concourse.bass_utils import run_bass_kernel_spmd

B, S, D, H = 16, 128, 768, 768
P = S * (S + 1) // 2  # 8256 upper-triangular pairs
NCORES = 8
BPC = B // NCORES  # batches per core
NFT = 64  # full pair tiles of 128 pairs (covering 8192 pairs)
NT = NFT + 1  # plus one 64-pair tail tile
TAIL = P - NFT * S  # 64
GRP = 8  # pair tiles per output staging group (1024 pairs, ~1.57MB fp16)
NGRP = NFT // GRP  # 8 full groups per batch

F32 = mybir.dt.float32
F16 = mybir.dt.float16
FP8 = mybir.dt.float8e4

TANH = mybir.ActivationFunctionType.Tanh

SEL_CH = 8  # selector tiles per DRAM chunk (1KB-contiguous lines)
NSC = NFT // SEL_CH  # 8 full-tile chunks; the tail tile rides sel_t


def _build_nc(repeat: int = 1) -> bass.Bass:
    nc = bacc.Bacc(
        "TRN2", target_bir_lowering=False, debug=False, num_devices=NCORES
    )

    NCH = D // 128  # 6 k-chunks

    # hid host-packed [128(k), BPC, NCH, S]: one load, 3KB lines.
    hidt = nc.declare_dram_parameter("hidt", [128, BPC, NCH, S], F16, isOutput=False)
    # W host-packed [NCH, 128(k), 2(half), H]: per-chunk loads, 3KB lines.
    w = nc.declare_dram_parameter("w", [NCH, 128, 2, H], F16, isOutput=False)
    bias = nc.declare_dram_parameter("bias", [1, H], F16, isOutput=False)
    # selectors chunk-major for contiguous direct loads.
    sel_i = nc.declare_dram_parameter("sel_i", [NSC, S, SEL_CH, S], FP8, isOutput=False)
    sel_j = nc.declare_dram_parameter("sel_j", [NSC, S, SEL_CH, S], FP8, isOutput=False)
    sel_t = nc.declare_dram_parameter("sel_t", [S, 2, S], FP8, isOutput=False)
    out = nc.declare_dram_parameter("out", [BPC, P, H], F16, isOutput=True)

    with tile.TileContext(nc) as tc, ExitStack() as ctx:
        consts = ctx.enter_context(tc.tile_pool(name="consts", bufs=1))
        acts = ctx.enter_context(tc.tile_pool(name="acts", bufs=2))
        outs = ctx.enter_context(tc.tile_pool(name="outs", bufs=2))
        # One shared PSUM pool: every tile is [128, 2048] f32 = 4 banks;
        # bufs=2 -> all 8 banks. start=True clears a whole PSUM bank, so the
        # two 768-wide sub-tiles must not share a bank: offsets 0 and 1024.
        psum = ctx.enter_context(tc.tile_pool(name="psum", bufs=2, space="PSUM"))

        # ---- constants computed on-engine FIRST: the PE warmup matmuls
        # depend on warm_w, and anything queued ahead of these memsets on
        # the GpSimd queue would stall the whole PE stream.
        ones_sb = consts.tile([1, 128], F16, name="ones")
        nc.gpsimd.memset(ones_sb[:], 1.0)
        warm_w = consts.tile([128, 128], F16, name="warm_w")
        nc.gpsimd.memset(warm_w[:], 0.125)

        # ---- SBUF tiles
        selib_i = consts.tile([S, NFT, S], FP8, name="selib_i")
        selib_j = consts.tile([S, NFT, S], FP8, name="selib_j")
        selib_t = consts.tile([S, 2, S], FP8, name="selib_t")
        seli_mm = consts.tile([S, NT, S], F16, name="seli_mm")
        selj_mm = consts.tile([S, NT, S], F16, name="selj_mm")
        w_sb = consts.tile([128, NCH, 2, H], F16, name="w_sb")
        bias_sb = consts.tile([1, H], F16, name="bias_sb")
        hid_sb = consts.tile([128, BPC, NCH, S], F16, name="hid_sb")

        # ---- loads, need-ordered across the two HWDGE rings. The early
        # loads share the ~358 GB/s per-core HBM cap (the sync ring
        # sustains ~205 GB/s, the scalar ring only ~150-170, so the
        # scalar ring carries fewer bytes). Combined need order: hid+Wc0
        # gate the first stage-A matmul; selector group 0 (0.26MB) rides
        # early for its DVE cast; remaining W chunks in consumption
        # order; then the bulk selectors. All scalar-ring (ACT-issued)
        # dma_starts complete their issue long before the first ACTIVATE.
        nc.sync.dma_start(w_sb[:, 0, :, :], w[0])
        nc.scalar.dma_start(hid_sb[:], hidt[:])
        nc.sync.dma_start(w_sb[:, 1, :, :], w[1])
        nc.scalar.dma_start(w_sb[:, 2, :, :], w[2])
        nc.sync.dma_start(selib_i[:, 0:SEL_CH, :], sel_i[0])
        nc.sync.dma_start(selib_j[:, 0:SEL_CH, :], sel_j[0])
        nc.scalar.dma_start(w_sb[:, 4, :, :], w[4])
        nc.sync.dma_start(w_sb[:, 3, :, :], w[3])
        nc.sync.dma_start(w_sb[:, 5, :, :], w[5])
        nc.sync.dma_start(bias_sb[:], bias[:])
        # bulk selectors: 8-tile chunks on the scalar ring (the sync ring
        # carries all output stores).
        for sc in range(1, NSC):
            nc.scalar.dma_start(
                selib_i[:, sc * SEL_CH : (sc + 1) * SEL_CH, :], sel_i[sc]
            )
            nc.scalar.dma_start(
                selib_j[:, sc * SEL_CH : (sc + 1) * SEL_CH, :], sel_j[sc]
            )
        nc.scalar.dma_start(selib_t[:], sel_t[:])

        # group-0 selector casts on the DVE (1.2us each, finished well
        # before the stage-A copies are ready, so no FIFO collision); all
        # later groups + the tail on GPSIMD (~3.6us each, serial, but the
        # deadline for group g is ~1.5us * 8g after the stream starts).
        nc.vector.tensor_copy(seli_mm[:, 0:SEL_CH, :], selib_i[:, 0:SEL_CH, :])
        nc.vector.tensor_copy(selj_mm[:, 0:SEL_CH, :], selib_j[:, 0:SEL_CH, :])
        for sc in range(1, NSC):
            sl = slice(sc * SEL_CH, (sc + 1) * SEL_CH)
            nc.gpsimd.tensor_copy(seli_mm[:, sl, :], selib_i[:, sl, :])
            nc.gpsimd.tensor_copy(selj_mm[:, sl, :], selib_j[:, sl, :])
            if sc == 3:
                # tail selector cast before chunk 4: the tail tile is
                # processed mid-stream at group 3 (~tile 28's slot).
                nc.gpsimd.tensor_copy(
                    seli_mm[:, NFT : NFT + 1, :], selib_t[:, 0:1, :]
                )
                nc.gpsimd.tensor_copy(
                    selj_mm[:, NFT : NFT + 1, :], selib_t[:, 1:2, :]
                )

        # PE warmup: ~1.5us of junk matmuls so the HAM clock-gate opens
        # before the real matmuls arrive (cold PE runs at half rate).
        warm_ps = psum.tile([128, 2048], F32, name="warm_ps", tag="ps")
        for k in range(28):
            nc.tensor.matmul(
                warm_ps[:, 0:128], lhsT=warm_w[:], rhs=warm_w[:], start=True, stop=True
            )

        for rep in range(repeat):
            # ---- stage A: projections for both batches --------------------
            # Both batches' PSUM tiles live at once (8 banks) and the chunk
            # matmuls interleave across batches, so the accumulation chases
            # the per-chunk W loads down the rings.
            pps = [
                psum.tile([128, 2048], F32, name=f"pp_{rep}_{bb}", tag="ps")
                for bb in range(BPC)
            ]
            # W1 (proj_i) matmuls lead within each chunk so the proj_i
            # accumulation chain finishes first; its PSUM->SBUF copies then
            # start while the W2/bias matmuls are still on the PE.
            for c in range(NCH - 1):
                first = c == 0
                for bb in range(BPC):
                    pp = pps[bb]
                    hT = hid_sb[:, bb, c, :]
                    nc.tensor.matmul(
                        pp[:, 0:512], lhsT=hT, rhs=w_sb[:, c, 0, 0:512],
                        start=first, stop=False,
                    )
                    nc.tensor.matmul(
                        pp[:, 512:768], lhsT=hT, rhs=w_sb[:, c, 0, 512:768],
                        start=first, stop=False,
                    )
                for bb in range(BPC):
                    pp = pps[bb]
                    hT = hid_sb[:, bb, c, :]
                    nc.tensor.matmul(
                        pp[:, 1024:1536], lhsT=hT, rhs=w_sb[:, c, 1, 0:512],
                        start=first, stop=False,
                    )
                    nc.tensor.matmul(
                        pp[:, 1536:1792], lhsT=hT, rhs=w_sb[:, c, 1, 512:768],
                        start=first, stop=False,
                    )
            c = NCH - 1
            pis, pjs = [], []
            for bb in range(BPC):
                pp = pps[bb]
                hT = hid_sb[:, bb, c, :]
                nc.tensor.matmul(
                    pp[:, 0:512], lhsT=hT, rhs=w_sb[:, c, 0, 0:512],
                    start=False, stop=True,
                )
                nc.tensor.matmul(
                    pp[:, 512:768], lhsT=hT, rhs=w_sb[:, c, 0, 512:768],
                    start=False, stop=True,
                )
                pi = acts.tile([S, H], F16, name=f"pi_{rep}_{bb}")
                nc.vector.tensor_copy(pi[:], pp[:, 0:768])
                pis.append(pi)
            for bb in range(BPC):
                pp = pps[bb]
                hT = hid_sb[:, bb, c, :]
                nc.tensor.matmul(
                    pp[:, 1024:1536], lhsT=hT, rhs=w_sb[:, c, 1, 0:512],
                    start=False, stop=False,
                )
                nc.tensor.matmul(
                    pp[:, 1536:1792], lhsT=hT, rhs=w_sb[:, c, 1, 512:768],
                    start=False, stop=False,
                )
                # bias folded in as a K=1 matmul of ones.T @ b
                nc.tensor.matmul(
                    pp[:, 1024:1536], lhsT=ones_sb[:], rhs=bias_sb[:, 0:512],
                    start=False, stop=True,
                )
                nc.tensor.matmul(
                    pp[:, 1536:1792], lhsT=ones_sb[:], rhs=bias_sb[:, 512:768],
                    start=False, stop=True,
                )
                pj = acts.tile([S, H], F16, name=f"pj_{rep}_{bb}")
                nc.scalar.copy(pj[:], pp[:, 1024:1792])
                pjs.append(pj)

            # ---- stage B: pair tiles, both batches per selector load ------
            for g in range(NGRP):
                t0 = g * GRP
                og2 = outs.tile(
                    [128, BPC, GRP, H], F16, name=f"og_{rep}_{g}", tag="og"
                )
                for tt in range(GRP):
                    t = t0 + tt
                    pq = psum.tile(
                        [128, 2048], F32, name=f"pq_{rep}_{t}", tag="ps"
                    )
                    for sel, prs, st in (
                        (seli_mm, pis, True),
                        (selj_mm, pjs, False),
                    ):
                        nc.tensor.matmul(
                            pq[:, 0:512], lhsT=sel[:, t, :],
                            rhs=prs[0][:, 0:512], start=st, stop=not st,
                        )
                        nc.tensor.matmul(
                            pq[:, 512:768], lhsT=sel[:, t, :],
                            rhs=prs[0][:, 512:768], start=st, stop=not st,
                        )
                        nc.tensor.matmul(
                            pq[:, 1024:1536], lhsT=sel[:, t, :],
                            rhs=prs[1][:, 0:512], start=st, stop=not st,
                        )
                        nc.tensor.matmul(
                            pq[:, 1536:1792], lhsT=sel[:, t, :],
                            rhs=prs[1][:, 512:768], start=st, stop=not st,
                        )
                    nc.scalar.activation(
                        og2[:, :, tt, :],
                        pq.rearrange("p (t x) -> p t x", t=2)[:, :, 0:768],
                        TANH,
                    )
                    r0 = t0 * S
                    dsts = [
                        out[bb, r0 : r0 + GRP * S, :].rearrange(
                            "(p g) h -> p g h", p=128
                        )
                        for bb in range(BPC)
                    ]
                    if g == NGRP - 2 and tt in (1, 3, 5, 7):
                        # group 6: 2-tile (0.39MB) pieces as activations land
                        for bb in range(BPC):
                            nc.sync.dma_start(
                                dsts[bb][:, tt - 1 : tt + 1, :],
                                og2[:, bb, tt - 1 : tt + 1, :],
                            )
                    elif g == NGRP - 1 and (tt in (1, 3, 5) or tt >= 6):
                        # group 7: 2-tile pieces, then per-tile at the very
                        # end so the post-compute drain is one tile deep.
                        sl = slice(tt - 1, tt + 1) if tt < 6 else slice(tt, tt + 1)
                        for bb in range(BPC):
                            nc.sync.dma_start(
                                dsts[bb][:, sl, :], og2[:, bb, sl, :]
                            )
                if g < NGRP - 2:
                    for bb in range(BPC):
                        nc.sync.dma_start(dsts[bb], og2[:, bb, :, :])
                if g == 3:
                    # ---- tail: 64 pairs via selector tile NFT, processed
                    # mid-stream (after the bulk selector load has landed)
                    # so its ACT/store latency hides ----------------------
                    pqt = psum.tile([128, 2048], F32, name=f"pqt_{rep}", tag="ps")
                    for sel, prs, st in (
                        (seli_mm, pis, True),
                        (selj_mm, pjs, False),
                    ):
                        nc.tensor.matmul(
                            pqt[:, 0:512], lhsT=sel[:, NFT, :],
                            rhs=prs[0][:, 0:512], start=st, stop=not st,
                        )
                        nc.tensor.matmul(
                            pqt[:, 512:768], lhsT=sel[:, NFT, :],
                            rhs=prs[0][:, 512:768], start=st, stop=not st,
                        )
                        nc.tensor.matmul(
                            pqt[:, 1024:1536], lhsT=sel[:, NFT, :],
                            rhs=prs[1][:, 0:512], start=st, stop=not st,
                        )
                        nc.tensor.matmul(
                            pqt[:, 1536:1792], lhsT=sel[:, NFT, :],
                            rhs=prs[1][:, 512:768], start=st, stop=not st,
                        )
                    og2t = acts.tile([128, BPC, H], F16, name=f"ogt_{rep}")
                    nc.scalar.activation(
                        og2t[0:TAIL, :, :],
                        pqt.rearrange("p (t x) -> p t x", t=2)[0:TAIL, :, 0:768],
                        TANH,
                    )
                    for bb in range(BPC):
                        nc.sync.dma_start(
                            out[bb, NFT * S : P, :], og2t[0:TAIL, bb, :]
                        )

    nc.compile()
    return nc


_NC_CACHE: dict[int, bass.Bass] = {}
LAST_RESULTS = None  # BassKernelResults of the most recent kernel() call


def _get_nc(repeat: int = 1) -> bass.Bass:
    if repeat not in _NC_CACHE:
        _NC_CACHE[repeat] = _build_nc(repeat)
    return _NC_CACHE[repeat]


_SEL_CACHE = None


def _selectors() -> tuple[np.ndarray, np.ndarray, np.ndarray]:
    """0/1 selector matrices, fp8 (exact), chunk-major [NSC, S, SEL_CH, S]
    (chunk, token k, tile-within-chunk, column m) so each chunk load has
    2KB-contiguous per-partition lines. Tile t<64 column m selects pair
    1024*(t//8) + 8*m + (t%8); the tail selectors (pair 8192+m in column
    m<64) ride sel_t[S, 2, S] = (k, {i,j}, m)."""
    global _SEL_CACHE
    if _SEL_CACHE is not None:
        return _SEL_CACHE
    import ml_dtypes

    ii, jj = np.triu_indices(S)
    sel_i = np.zeros((NSC, S, SEL_CH, S), dtype=np.float32)
    sel_j = np.zeros((NSC, S, SEL_CH, S), dtype=np.float32)
    m = np.arange(S)
    for t in range(NFT):
        pr = 1024 * (t // 8) + 8 * m + (t % 8)
        sel_i[t // SEL_CH, ii[pr], t % SEL_CH, m] = 1.0
        sel_j[t // SEL_CH, jj[pr], t % SEL_CH, m] = 1.0
    sel_t = np.zeros((S, 2, S), dtype=np.float32)
    mt = np.arange(TAIL)
    pr = NFT * S + mt
    sel_t[ii[pr], 0, mt] = 1.0
    sel_t[jj[pr], 1, mt] = 1.0
    _SEL_CACHE = (
        sel_i.astype(ml_dtypes.float8_e4m3),
        sel_j.astype(ml_dtypes.float8_e4m3),
        sel_t.astype(ml_dtypes.float8_e4m3),
    )
    return _SEL_CACHE


def kernel(hidden: np.ndarray, W: np.ndarray, b: np.ndarray) -> np.ndarray:
    hidden = np.asarray(hidden, dtype=np.float32)
    W = np.asarray(W, dtype=np.float32)
    b = np.asarray(b, dtype=np.float32)

    sel_i, sel_j, sel_t = _selectors()
    # hidden packed per core to [128(k), BPC, NCH, S]: per-partition lines
    # are BPC*NCH*S*2 = 3KB contiguous, loaded in one dma_start.
    # hidden[b, s, d] with d = c*128 + k  ->  hidt[k, b, c, s]
    hidt = np.ascontiguousarray(
        hidden.transpose(2, 0, 1)  # [D, B, S]
        .reshape(D // 128, 128, B, S)
        .transpose(1, 2, 0, 3)  # [128(k), B, NCH, S]
        .astype(np.float16)
    )
    # W packed to [NCH, 128(k), 2(half), H]: one 3KB-line load per chunk.
    # W[d, h] with d = half*768 + c*128 + k -> w[c, k, half, h]
    w16 = np.ascontiguousarray(
        W.reshape(2, D // 128, 128, H).transpose(1, 2, 0, 3).astype(np.float16)
    )
    b16 = b.astype(np.float16).reshape(1, H)

    nc = _get_nc()
    in_maps = []
    for c in range(NCORES):
        in_maps.append(
            {
                "hidt": np.ascontiguousarray(hidt[:, c * BPC : (c + 1) * BPC]),
                "w": w16,
                "bias": b16,
                "sel_i": sel_i,
                "sel_j": sel_j,
                "sel_t": sel_t,
            }
        )
    res = run_bass_kernel_spmd(nc, in_maps, list(range(NCORES)))
    global LAST_RESULTS
    LAST_RESULTS = res
    out = np.concatenate([res.results[c]["out"] for c in range(NCORES)], axis=0)
    return out.astype(np.float32)


# revision 49
# speedup vs baseline: 1.0273x; 1.0224x over previous
"""Trainium2 Bass kernel for nn_ConcatHandshaking.

Computes out[b, p, :] = tanh(proj_i[b, ii[p], :] + proj_j[b, jj[p], :])
where proj_i = hidden @ W[:D], proj_j = hidden @ W[D:] + bias, and (ii, jj)
are the upper-triangular token pairs of a length-S sequence.

Sharding: data-parallel over batch. B=16 batches -> 2 per core on 8 cores.

All on-device data is fp16 (host casts inputs, output cast back to f32 on
host; fp16 keeps max rel err ~2.6e-3 vs the 2e-2 gate). This halves the
dominant HBM traffic (the 8256x768 output per batch) versus f32.

The kernel is ACT-bound: tanh only runs on ScalarE at 1 elem/cycle/lane,
so the 2*8256*768 = 12.68M output elems per core cost >=82.5us of ACT
busy (plus ~260ns/instr overhead, 66 instrs -> ~97us). Everything else
(PE ~108us busy incl. LDWEIGHTS, stores ~71us at 358GB/s) must hide
under the ACT stream, so the schedule optimizes (a) time-to-first-
ACTIVATE and (b) the post-ACT store drain. There is also an ~8us fixed
NEFF preamble (engine init + instruction fetch) before any DMA flows.

Per-core pipeline:
  Stage A: hidden arrives host-packed as [128, BPC, D/128, S] (3KB DMA
           lines, one load); W host-packed per k-chunk as [D/128, 128, 2, H]
           so each chunk (both halves) is one 3KB-line load, issued in
           need-order alternating across the two HWDGE rings. fp16
           matmuls hidT.T x W-half accumulate both batches' projections
           in PSUM f32 chunk-by-chunk as the W chunks land; bias folds in
           as a K=1 ones-vector matmul; proj_i drains on DVE, proj_j on
           ScalarE (both idle before the tanh stream starts).
  Stage B: pair axis (P=8256) split into 64 full tiles of 128 pairs plus a
           64-pair tail. Tile t (t<64) holds pairs 1024*(t//8) + 8*m + (t%8),
           m=0..127, so each PSUM partition m of an 8-tile output group
           holds 8 CONSECUTIVE out rows -> 12KB contiguous DMA
           descriptors (the store ring must sustain the ACT production
           rate of ~248 GB/s or the drain tail grows). Each tile is
           processed for BOTH batches under one selector weight load
           (stationary reuse: 1 LDWEIGHTS per selector per tile instead
           of per batch), accumulating
           selI.T @ proj_i + selJ.T @ proj_j into one 4-bank PSUM tile
           (batch0 at cols 0:768, batch1 at 1024:1792). One ScalarE tanh
           drains both batches' rows into an fp16 staging group; groups
           0..5 are stored with ~0.79MB per-batch DMAs, groups 6..7 in
           2-tile (0.39MB) pieces as activations land, so the
           post-compute drain is short.

Selectors are fp8 (0/1 exact) in DRAM, CHUNK-MAJOR [4, S, 16, S]: a
16-tile chunk load has 2KB-contiguous per-partition lines ([S, NT, S]
made the DMA read 128-byte DRAM fragments at ~70 GB/s and gated the
ramp; dma_start_transpose of an fp16 pair-major layout was even worse,
16640 256-byte xbar descriptors clogging the ring to 39us). The two
tail-tile selectors ride one [S, 2, S] tensor. Casts to fp16 run on
GPSIMD (otherwise idle; ~3.4 cyc/elem is plenty ahead of use), with
chunk 0 split per-GROUP so tile 0's weights are ready early. The PE
must NOT consume fp8 selectors directly: a mixed fp8-lhsT x fp16-rhs
matmul runs at HALF rate on HW (450ns vs 216ns per 512 cols). The DVE
runs ONLY the two proj_i PSUM->SBUF copies: the Tile scheduler
dispatches by operand readiness, not program order, so anything else
queued there (e.g. casts) jumps ahead of the copies and stalls tile 0.
All stores go on the sync (SP) HWDGE ring; every scalar-ring
(ACT-issued) dma_start is emitted before the first ACTIVATE so the ACT
queue never stalls on DMA issue mid-stream.
"""

import sys

if "/opt/trn_rl_repo" not in sys.path:
    sys.path.insert(0, "/opt/trn_rl_repo")

from contextlib import ExitStack

import numpy as np

import concourse.bass as bass
import concourse.mybir as mybir
import concourse.tile as tile
from concourse import bacc
from concourse.bass_utils import run_bass_kernel_spmd

B, S, D, H = 16, 128, 768, 768
P = S * (S + 1) // 2  # 8256 upper-triangular pairs
NCORES = 8
BPC = B // NCORES  # batches per core
NFT = 64  # full pair tiles of 128 pairs (covering 8192 pairs)
NT = NFT + 1  # plus one 64-pair tail tile
TAIL = P - NFT * S  # 64
GRP = 8  # pair tiles per output staging group (1024 pairs, ~1.57MB fp16)
NGRP = NFT // GRP  # 8 full groups per batch

F32 = mybir.dt.float32
F16 = mybir.dt.float16
FP8 = mybir.dt.float8e4

TANH = mybir.ActivationFunctionType.Tanh

SEL_CH = 8  # selector tiles per DRAM chunk (1KB-contiguous lines)
NSC = NFT // SEL_CH  # 8 full-tile chunks; the tail tile rides sel_t


def _build_nc(repeat: int = 1) -> bass.Bass:
    nc = bacc.Bacc(
        "TRN2", target_bir_lowering=False, debug=False, num_devices=NCORES
    )

    NCH = D // 128  # 6 k-chunks

    # hid host-packed [128(k), BPC, NCH, S]: one load, 3KB lines.
    hidt = nc.declare_dram_parameter("hidt", [128, BPC, NCH, S], F16, isOutput=False)
    # W host-packed [NCH, 128(k), 2(half), H]: per-chunk loads, 3KB lines.
    w = nc.declare_dram_parameter("w", [NCH, 128, 2, H], F16, isOutput=False)
    bias = nc.declare_dram_parameter("bias", [1, H], F16, isOutput=False)
    # selectors chunk-major for contiguous direct loads.
    sel_i = nc.declare_dram_parameter("sel_i", [NSC, S, SEL_CH, S], FP8, isOutput=False)
    sel_j = nc.declare_dram_parameter("sel_j", [NSC, S, SEL_CH, S], FP8, isOutput=False)
    sel_t = nc.declare_dram_parameter("sel_t", [S, 2, S], FP8, isOutput=False)
    out = nc.declare_dram_parameter("out", [BPC, P, H], F16, isOutput=True)

    with tile.TileContext(nc) as tc, ExitStack() as ctx:
        consts = ctx.enter_context(tc.tile_pool(name="consts", bufs=1))
        acts = ctx.enter_context(tc.tile_pool(name="acts", bufs=2))
        outs = ctx.enter_context(tc.tile_pool(name="outs", bufs=2))
        # One shared PSUM pool: every tile is [128, 2048] f32 = 4 banks;
        # bufs=2 -> all 8 banks. start=True clears a whole PSUM bank, so the
        # two 768-wide sub-tiles must not share a bank: offsets 0 and 1024.
        psum = ctx.enter_context(tc.tile_pool(name="psum", bufs=2, space="PSUM"))

        # ---- constants computed on-engine FIRST: the PE warmup matmuls
        # depend on warm_w, and anything queued ahead of these memsets on
        # the GpSimd queue would stall the whole PE stream.
        ones_sb = consts.tile([1, 128], F16, name="ones")
        nc.gpsimd.memset(ones_sb[:], 1.0)
        warm_w = consts.tile([128, 128], F16, name="warm_w")
        nc.gpsimd.memset(warm_w[:], 0.125)

        # ---- SBUF tiles
        selib_i = consts.tile([S, NFT, S], FP8, name="selib_i")
        selib_j = consts.tile([S, NFT, S], FP8, name="selib_j")
        selib_t = consts.tile([S, 2, S], FP8, name="selib_t")
        seli_mm = consts.tile([S, NT, S], F16, name="seli_mm")
        selj_mm = consts.tile([S, NT, S], F16, name="selj_mm")
        w_sb = consts.tile([128, NCH, 2, H], F16, name="w_sb")
        bias_sb = consts.tile([1, H], F16, name="bias_sb")
        hid_sb = consts.tile([128, BPC, NCH, S], F16, name="hid_sb")

        # ---- loads, need-ordered across the two HWDGE rings. The early
        # loads share the ~358 GB/s per-core HBM cap (the sync ring
        # sustains ~205 GB/s, the scalar ring only ~150-170, so the
        # scalar ring carries fewer bytes). Combined need order: hid+Wc0
        # gate the first stage-A matmul; selector group 0 (0.26MB) rides
        # early for its DVE cast; remaining W chunks in consumption
        # order; then the bulk selectors. All scalar-ring (ACT-issued)
        # dma_starts complete their issue long before the first ACTIVATE.
        nc.sync.dma_start(w_sb[:, 0, :, :], w[0])
        nc.scalar.dma_start(hid_sb[:], hidt[:])
        nc.sync.dma_start(selib_i[:, 0:SEL_CH, :], sel_i[0])
        nc.sync.dma_start(selib_j[:, 0:SEL_CH, :], sel_j[0])
        nc.scalar.dma_start(w_sb[:, 1, :, :], w[1])
        nc.sync.dma_start(w_sb[:, 2, :, :], w[2])
        nc.scalar.dma_start(w_sb[:, 3, :, :], w[3])
        nc.sync.dma_start(w_sb[:, 4, :, :], w[4])
        nc.sync.dma_start(w_sb[:, 5, :, :], w[5])
        nc.sync.dma_start(bias_sb[:], bias[:])
        # bulk selectors: 8-tile chunks on the sync ring ahead of the
        # first group store (which is only issued ~30us in).
        for sc in range(1, NSC):
            nc.sync.dma_start(
                selib_i[:, sc * SEL_CH : (sc + 1) * SEL_CH, :], sel_i[sc]
            )
            nc.sync.dma_start(
                selib_j[:, sc * SEL_CH : (sc + 1) * SEL_CH, :], sel_j[sc]
            )
        nc.sync.dma_start(selib_t[:], sel_t[:])

        # group-0 selector casts on the DVE (1.2us each, finished well
        # before the stage-A copies are ready, so no FIFO collision); all
        # later groups + the tail on GPSIMD (~3.6us each, serial, but the
        # deadline for group g is ~1.5us * 8g after the stream starts).
        nc.vector.tensor_copy(seli_mm[:, 0:SEL_CH, :], selib_i[:, 0:SEL_CH, :])
        nc.vector.tensor_copy(selj_mm[:, 0:SEL_CH, :], selib_j[:, 0:SEL_CH, :])
        for sc in range(1, NSC):
            sl = slice(sc * SEL_CH, (sc + 1) * SEL_CH)
            nc.gpsimd.tensor_copy(seli_mm[:, sl, :], selib_i[:, sl, :])
            nc.gpsimd.tensor_copy(selj_mm[:, sl, :], selib_j[:, sl, :])
            if sc == 3:
                # tail selector cast before chunk 4: the tail tile is
                # processed mid-stream at group 3 (~tile 28's slot).
                nc.gpsimd.tensor_copy(
                    seli_mm[:, NFT : NFT + 1, :], selib_t[:, 0:1, :]
                )
                nc.gpsimd.tensor_copy(
                    selj_mm[:, NFT : NFT + 1, :], selib_t[:, 1:2, :]
                )

        # PE warmup: ~1.5us of junk matmuls so the HAM clock-gate opens
        # before the real matmuls arrive (cold PE runs at half rate).
        warm_ps = psum.tile([128, 2048], F32, name="warm_ps", tag="ps")
        for k in range(28):
            nc.tensor.matmul(
                warm_ps[:, 0:128], lhsT=warm_w[:], rhs=warm_w[:], start=True, stop=True
            )

        for rep in range(repeat):
            # ---- stage A: projections for both batches --------------------
            # Both batches' PSUM tiles live at once (8 banks) and the chunk
            # matmuls interleave across batches, so the accumulation chases
            # the per-chunk W loads down the rings.
            pps = [
                psum.tile([128, 2048], F32, name=f"pp_{rep}_{bb}", tag="ps")
                for bb in range(BPC)
            ]
            # W1 (proj_i) matmuls lead within each chunk so the proj_i
            # accumulation chain finishes first; its PSUM->SBUF copies then
            # start while the W2/bias matmuls are still on the PE.
            for c in range(NCH - 1):
                first = c == 0
                for bb in range(BPC):
                    pp = pps[bb]
                    hT = hid_sb[:, bb, c, :]
                    nc.tensor.matmul(
                        pp[:, 0:512], lhsT=hT, rhs=w_sb[:, c, 0, 0:512],
                        start=first, stop=False,
                    )
                    nc.tensor.matmul(
                        pp[:, 512:768], lhsT=hT, rhs=w_sb[:, c, 0, 512:768],
                        start=first, stop=False,
                    )
                for bb in range(BPC):
                    pp = pps[bb]
                    hT = hid_sb[:, bb, c, :]
                    nc.tensor.matmul(
                        pp[:, 1024:1536], lhsT=hT, rhs=w_sb[:, c, 1, 0:512],
                        start=first, stop=False,
                    )
                    nc.tensor.matmul(
                        pp[:, 1536:1792], lhsT=hT, rhs=w_sb[:, c, 1, 512:768],
                        start=first, stop=False,
                    )
            c = NCH - 1
            pis, pjs = [], []
            for bb in range(BPC):
                pp = pps[bb]
                hT = hid_sb[:, bb, c, :]
                nc.tensor.matmul(
                    pp[:, 0:512], lhsT=hT, rhs=w_sb[:, c, 0, 0:512],
                    start=False, stop=True,
                )
                nc.tensor.matmul(
                    pp[:, 512:768], lhsT=hT, rhs=w_sb[:, c, 0, 512:768],
                    start=False, stop=True,
                )
                pi = acts.tile([S, H], F16, name=f"pi_{rep}_{bb}")
                nc.vector.tensor_copy(pi[:], pp[:, 0:768])
                pis.append(pi)
            for bb in range(BPC):
                pp = pps[bb]
                hT = hid_sb[:, bb, c, :]
                nc.tensor.matmul(
                    pp[:, 1024:1536], lhsT=hT, rhs=w_sb[:, c, 1, 0:512],
                    start=False, stop=False,
                )
                nc.tensor.matmul(
                    pp[:, 1536:1792], lhsT=hT, rhs=w_sb[:, c, 1, 512:768],
                    start=False, stop=False,
                )
                # bias folded in as a K=1 matmul of ones.T @ b
                nc.tensor.matmul(
                    pp[:, 1024:1536], lhsT=ones_sb[:], rhs=bias_sb[:, 0:512],
                    start=False, stop=True,
                )
                nc.tensor.matmul(
                    pp[:, 1536:1792], lhsT=ones_sb[:], rhs=bias_sb[:, 512:768],
                    start=False, stop=True,
                )
                pj = acts.tile([S, H], F16, name=f"pj_{rep}_{bb}")
                nc.scalar.copy(pj[:], pp[:, 1024:1792])
                pjs.append(pj)

            # ---- stage B: pair tiles, both batches per selector load ------
            for g in range(NGRP):
                t0 = g * GRP
                og2 = outs.tile(
                    [128, BPC, GRP, H], F16, name=f"og_{rep}_{g}", tag="og"
                )
                for tt in range(GRP):
                    t = t0 + tt
                    pq = psum.tile(
                        [128, 2048], F32, name=f"pq_{rep}_{t}", tag="ps"
                    )
                    for sel, prs, st in (
                        (seli_mm, pis, True),
                        (selj_mm, pjs, False),
                    ):
                        nc.tensor.matmul(
                            pq[:, 0:512], lhsT=sel[:, t, :],
                            rhs=prs[0][:, 0:512], start=st, stop=not st,
                        )
                        nc.tensor.matmul(
                            pq[:, 512:768], lhsT=sel[:, t, :],
                            rhs=prs[0][:, 512:768], start=st, stop=not st,
                        )
                        nc.tensor.matmul(
                            pq[:, 1024:1536], lhsT=sel[:, t, :],
                            rhs=prs[1][:, 0:512], start=st, stop=not st,
                        )
                        nc.tensor.matmul(
                            pq[:, 1536:1792], lhsT=sel[:, t, :],
                            rhs=prs[1][:, 512:768], start=st, stop=not st,
                        )
                    nc.scalar.activation(
                        og2[:, :, tt, :],
                        pq.rearrange("p (t x) -> p t x", t=2)[:, :, 0:768],
                        TANH,
                    )
                    r0 = t0 * S
                    dsts = [
                        out[bb, r0 : r0 + GRP * S, :].rearrange(
                            "(p g) h -> p g h", p=128
                        )
                        for bb in range(BPC)
                    ]
                    if g == NGRP - 2 and tt in (1, 3, 5, 7):
                        # group 6: 2-tile (0.39MB) pieces as activations land
                        for bb in range(BPC):
                            nc.sync.dma_start(
                                dsts[bb][:, tt - 1 : tt + 1, :],
                                og2[:, bb, tt - 1 : tt + 1, :],
                            )
                    elif g == NGRP - 1 and (tt in (1, 3, 5) or tt >= 6):
                        # group 7: 2-tile pieces, then per-tile at the very
                        # end so the post-compute drain is one tile deep.
                        sl = slice(tt - 1, tt + 1) if tt < 6 else slice(tt, tt + 1)
                        for bb in range(BPC):
                            nc.sync.dma_start(
                                dsts[bb][:, sl, :], og2[:, bb, sl, :]
                            )
                if g < NGRP - 2:
                    for bb in range(BPC):
                        nc.sync.dma_start(dsts[bb], og2[:, bb, :, :])
                if g == 3:
                    # ---- tail: 64 pairs via selector tile NFT, processed
                    # mid-stream (after the bulk selector load has landed)
                    # so its ACT/store latency hides ----------------------
                    pqt = psum.tile([128, 2048], F32, name=f"pqt_{rep}", tag="ps")
                    for sel, prs, st in (
                        (seli_mm, pis, True),
                        (selj_mm, pjs, False),
                    ):
                        nc.tensor.matmul(
                            pqt[:, 0:512], lhsT=sel[:, NFT, :],
                            rhs=prs[0][:, 0:512], start=st, stop=not st,
                        )
                        nc.tensor.matmul(
                            pqt[:, 512:768], lhsT=sel[:, NFT, :],
                            rhs=prs[0][:, 512:768], start=st, stop=not st,
                        )
                        nc.tensor.matmul(
                            pqt[:, 1024:1536], lhsT=sel[:, NFT, :],
                            rhs=prs[1][:, 0:512], start=st, stop=not st,
                        )
                        nc.tensor.matmul(
                            pqt[:, 1536:1792], lhsT=sel[:, NFT, :],
                            rhs=prs[1][:, 512:768], start=st, stop=not st,
                        )
                    og2t = acts.tile([128, BPC, H], F16, name=f"ogt_{rep}")
                    nc.scalar.activation(
                        og2t[0:TAIL, :, :],
                        pqt.rearrange("p (t x) -> p t x", t=2)[0:TAIL, :, 0:768],
                        TANH,
                    )
                    for bb in range(BPC):
                        nc.sync.dma_start(
                            out[bb, NFT * S : P, :], og2t[0:TAIL, bb, :]
                        )

    nc.compile()
    return nc


_NC_CACHE: dict[int, bass.Bass] = {}
LAST_RESULTS = None  # BassKernelResults of the most recent kernel() call


def _get_nc(repeat: int = 1) -> bass.Bass:
    if repeat not in _NC_CACHE:
        _NC_CACHE[repeat] = _build_nc(repeat)
    return _NC_CACHE[repeat]


_SEL_CACHE = None


def _selectors() -> tuple[np.ndarray, np.ndarray, np.ndarray]:
    """0/1 selector matrices, fp8 (exact), chunk-major [NSC, S, SEL_CH, S]
    (chunk, token k, tile-within-chunk, column m) so each chunk load has
    2KB-contiguous per-partition lines. Tile t<64 column m selects pair
    1024*(t//8) + 8*m + (t%8); the tail selectors (pair 8192+m in column
    m<64) ride sel_t[S, 2, S] = (k, {i,j}, m)."""
    global _SEL_CACHE
    if _SEL_CACHE is not None:
        return _SEL_CACHE
    import ml_dtypes

    ii, jj = np.triu_indices(S)
    sel_i = np.zeros((NSC, S, SEL_CH, S), dtype=np.float32)
    sel_j = np.zeros((NSC, S, SEL_CH, S), dtype=np.float32)
    m = np.arange(S)
    for t in range(NFT):
        pr = 1024 * (t // 8) + 8 * m + (t % 8)
        sel_i[t // SEL_CH, ii[pr], t % SEL_CH, m] = 1.0
        sel_j[t // SEL_CH, jj[pr], t % SEL_CH, m] = 1.0
    sel_t = np.zeros((S, 2, S), dtype=np.float32)
    mt = np.arange(TAIL)
    pr = NFT * S + mt
    sel_t[ii[pr], 0, mt] = 1.0
    sel_t[jj[pr], 1, mt] = 1.0
    _SEL_CACHE = (
        sel_i.astype(ml_dtypes.float8_e4m3),
        sel_j.astype(ml_dtypes.float8_e4m3),
        sel_t.astype(ml_dtypes.float8_e4m3),
    )
    return _SEL_CACHE


def kernel(hidden: np.ndarray, W: np.ndarray, b: np.ndarray) -> np.ndarray:
    hidden = np.asarray(hidden, dtype=np.float32)
    W = np.asarray(W, dtype=np.float32)
    b = np.asarray(b, dtype=np.float32)

    sel_i, sel_j, sel_t = _selectors()
    # hidden packed per core to [128(k), BPC, NCH, S]: per-partition lines
    # are BPC*NCH*S*2 = 3KB contiguous, loaded in one dma_start.
    # hidden[b, s, d] with d = c*128 + k  ->  hidt[k, b, c, s]
    hidt = np.ascontiguousarray(
        hidden.transpose(2, 0, 1)  # [D, B, S]
        .reshape(D // 128, 128, B, S)
        .transpose(1, 2, 0, 3)  # [128(k), B, NCH, S]
        .astype(np.float16)
    )
    # W packed to [NCH, 128(k), 2(half), H]: one 3KB-line load per chunk.
    # W[d, h] with d = half*768 + c*128 + k -> w[c, k, half, h]
    w16 = np.ascontiguousarray(
        W.reshape(2, D // 128, 128, H).transpose(1, 2, 0, 3).astype(np.float16)
    )
    b16 = b.astype(np.float16).reshape(1, H)

    nc = _get_nc()
    in_maps = []
    for c in range(NCORES):
        in_maps.append(
            {
                "hidt": np.ascontiguousarray(hidt[:, c * BPC : (c + 1) * BPC]),
                "w": w16,
                "bias": b16,
                "sel_i": sel_i,
                "sel_j": sel_j,
                "sel_t": sel_t,
            }
        )
    res = run_bass_kernel_spmd(nc, in_maps, list(range(NCORES)))
    global LAST_RESULTS
    LAST_RESULTS = res
    out = np.concatenate([res.results[c]["out"] for c in range(NCORES)], axis=0)
    return out.astype(np.float32)


# revision 52
# speedup vs baseline: 1.0322x; 1.0047x over previous
"""Trainium2 Bass kernel for nn_ConcatHandshaking.

Computes out[b, p, :] = tanh(proj_i[b, ii[p], :] + proj_j[b, jj[p], :])
where proj_i = hidden @ W[:D], proj_j = hidden @ W[D:] + bias, and (ii, jj)
are the upper-triangular token pairs of a length-S sequence.

Sharding: data-parallel over batch. B=16 batches -> 2 per core on 8 cores.

All on-device data is fp16 (host casts inputs, output cast back to f32 on
host; fp16 keeps max rel err ~2.6e-3 vs the 2e-2 gate). This halves the
dominant HBM traffic (the 8256x768 output per batch) versus f32.

The kernel is ACT-bound: tanh only runs on ScalarE at 1 elem/cycle/lane,
so the 2*8256*768 = 12.68M output elems per core cost >=82.5us of ACT
busy (plus ~260ns/instr overhead, 66 instrs -> ~97us). Everything else
(PE ~108us busy incl. LDWEIGHTS, stores ~71us at 358GB/s) must hide
under the ACT stream, so the schedule optimizes (a) time-to-first-
ACTIVATE and (b) the post-ACT store drain. There is also an ~8us fixed
NEFF preamble (engine init + instruction fetch) before any DMA flows.

Per-core pipeline:
  Stage A: hidden arrives host-packed as [128, BPC, D/128, S] (3KB DMA
           lines, one load); W host-packed per k-chunk as [D/128, 128, 2, H]
           so each chunk (both halves) is one 3KB-line load, issued in
           need-order alternating across the two HWDGE rings. fp16
           matmuls hidT.T x W-half accumulate both batches' projections
           in PSUM f32 chunk-by-chunk as the W chunks land; bias folds in
           as a K=1 ones-vector matmul; proj_i drains on DVE, proj_j on
           ScalarE (both idle before the tanh stream starts).
  Stage B: pair axis (P=8256) split into 64 full tiles of 128 pairs plus a
           64-pair tail. Tile t (t<64) holds pairs 1024*(t//8) + 8*m + (t%8),
           m=0..127, so each PSUM partition m of an 8-tile output group
           holds 8 CONSECUTIVE out rows -> 12KB contiguous DMA
           descriptors (the store ring must sustain the ACT production
           rate of ~248 GB/s or the drain tail grows). Each tile is
           processed for BOTH batches under one selector weight load
           (stationary reuse: 1 LDWEIGHTS per selector per tile instead
           of per batch), accumulating
           selI.T @ proj_i + selJ.T @ proj_j into one 4-bank PSUM tile
           (batch0 at cols 0:768, batch1 at 1024:1792). One ScalarE tanh
           drains both batches' rows into an fp16 staging group; groups
           0..5 are stored with ~0.79MB per-batch DMAs, groups 6..7 in
           2-tile (0.39MB) pieces as activations land, so the
           post-compute drain is short.

Selectors are fp8 (0/1 exact) in DRAM, CHUNK-MAJOR [4, S, 16, S]: a
16-tile chunk load has 2KB-contiguous per-partition lines ([S, NT, S]
made the DMA read 128-byte DRAM fragments at ~70 GB/s and gated the
ramp; dma_start_transpose of an fp16 pair-major layout was even worse,
16640 256-byte xbar descriptors clogging the ring to 39us). The two
tail-tile selectors ride one [S, 2, S] tensor. Casts to fp16 run on
GPSIMD (otherwise idle; ~3.4 cyc/elem is plenty ahead of use), with
chunk 0 split per-GROUP so tile 0's weights are ready early. The PE
must NOT consume fp8 selectors directly: a mixed fp8-lhsT x fp16-rhs
matmul runs at HALF rate on HW (450ns vs 216ns per 512 cols). The DVE
runs ONLY the two proj_i PSUM->SBUF copies: the Tile scheduler
dispatches by operand readiness, not program order, so anything else
queued there (e.g. casts) jumps ahead of the copies and stalls tile 0.
All stores go on the sync (SP) HWDGE ring; every scalar-ring
(ACT-issued) dma_start is emitted before the first ACTIVATE so the ACT
queue never stalls on DMA issue mid-stream.
"""

import sys

if "/opt/trn_rl_repo" not in sys.path:
    sys.path.insert(0, "/opt/trn_rl_repo")

from contextlib import ExitStack

import numpy as np

import concourse.bass as bass
import concourse.mybir as mybir
import concourse.tile as tile
from concourse import bacc
from concourse.bass_utils import run_bass_kernel_spmd

B, S, D, H = 16, 128, 768, 768
P = S * (S + 1) // 2  # 8256 upper-triangular pairs
NCORES = 8
BPC = B // NCORES  # batches per core
NFT = 64  # full pair tiles of 128 pairs (covering 8192 pairs)
NT = NFT + 1  # plus one 64-pair tail tile
TAIL = P - NFT * S  # 64
GRP = 8  # pair tiles per output staging group (1024 pairs, ~1.57MB fp16)
NGRP = NFT // GRP  # 8 full groups per batch

F32 = mybir.dt.float32
F16 = mybir.dt.float16
FP8 = mybir.dt.float8e4

TANH = mybir.ActivationFunctionType.Tanh

SEL_CH = 8  # selector tiles per DRAM chunk (1KB-contiguous lines)
NSC = NFT // SEL_CH  # 8 full-tile chunks; the tail tile rides sel_t


def _build_nc(repeat: int = 1) -> bass.Bass:
    nc = bacc.Bacc(
        "TRN2", target_bir_lowering=False, debug=False, num_devices=NCORES
    )

    NCH = D // 128  # 6 k-chunks

    # hid host-packed [128(k), BPC, NCH, S]: one load, 3KB lines.
    hidt = nc.declare_dram_parameter("hidt", [128, BPC, NCH, S], F16, isOutput=False)
    # W host-packed [NCH, 128(k), 2(half), H]: per-chunk loads, 3KB lines.
    w = nc.declare_dram_parameter("w", [NCH, 128, 2, H], F16, isOutput=False)
    bias = nc.declare_dram_parameter("bias", [1, H], F16, isOutput=False)
    # selectors chunk-major for contiguous direct loads.
    sel_i = nc.declare_dram_parameter("sel_i", [NSC, S, SEL_CH, S], FP8, isOutput=False)
    sel_j = nc.declare_dram_parameter("sel_j", [NSC, S, SEL_CH, S], FP8, isOutput=False)
    sel_t = nc.declare_dram_parameter("sel_t", [S, 2, S], FP8, isOutput=False)
    out = nc.declare_dram_parameter("out", [BPC, P, H], F16, isOutput=True)

    with tile.TileContext(nc) as tc, ExitStack() as ctx:
        consts = ctx.enter_context(tc.tile_pool(name="consts", bufs=1))
        acts = ctx.enter_context(tc.tile_pool(name="acts", bufs=2))
        outs = ctx.enter_context(tc.tile_pool(name="outs", bufs=2))
        # One shared PSUM pool: every tile is [128, 2048] f32 = 4 banks;
        # bufs=2 -> all 8 banks. start=True clears a whole PSUM bank, so the
        # two 768-wide sub-tiles must not share a bank: offsets 0 and 1024.
        psum = ctx.enter_context(tc.tile_pool(name="psum", bufs=2, space="PSUM"))

        # ---- constants computed on-engine FIRST: the PE warmup matmuls
        # depend on warm_w, and anything queued ahead of these memsets on
        # the GpSimd queue would stall the whole PE stream.
        ones_sb = consts.tile([1, 128], F16, name="ones")
        nc.gpsimd.memset(ones_sb[:], 1.0)
        warm_w = consts.tile([128, 128], F16, name="warm_w")
        nc.gpsimd.memset(warm_w[:], 0.125)

        # ---- SBUF tiles
        selib_i = consts.tile([S, NFT, S], FP8, name="selib_i")
        selib_j = consts.tile([S, NFT, S], FP8, name="selib_j")
        selib_t = consts.tile([S, 2, S], FP8, name="selib_t")
        seli_mm = consts.tile([S, NT, S], F16, name="seli_mm")
        selj_mm = consts.tile([S, NT, S], F16, name="selj_mm")
        w_sb = consts.tile([128, NCH, 2, H], F16, name="w_sb")
        bias_sb = consts.tile([1, H], F16, name="bias_sb")
        hid_sb = consts.tile([128, BPC, NCH, S], F16, name="hid_sb")

        # ---- loads, need-ordered across the two HWDGE rings. The early
        # loads share the ~358 GB/s per-core HBM cap (the sync ring
        # sustains ~205 GB/s, the scalar ring only ~150-170, so the
        # scalar ring carries fewer bytes). Combined need order: hid+Wc0
        # gate the first stage-A matmul; selector group 0 (0.26MB) rides
        # early for its DVE cast; remaining W chunks in consumption
        # order; then the bulk selectors. All scalar-ring (ACT-issued)
        # dma_starts complete their issue long before the first ACTIVATE.
        nc.sync.dma_start(w_sb[:, 0, :, :], w[0])
        nc.scalar.dma_start(hid_sb[:], hidt[:])
        nc.sync.dma_start(selib_i[:, 0:SEL_CH, :], sel_i[0])
        nc.scalar.dma_start(w_sb[:, 1, :, :], w[1])
        nc.sync.dma_start(w_sb[:, 2, :, :], w[2])
        nc.scalar.dma_start(w_sb[:, 3, :, :], w[3])
        nc.sync.dma_start(w_sb[:, 4, :, :], w[4])
        nc.sync.dma_start(selib_j[:, 0:SEL_CH, :], sel_j[0])
        nc.sync.dma_start(w_sb[:, 5, :, :], w[5])
        nc.sync.dma_start(bias_sb[:], bias[:])
        # bulk selectors: 8-tile chunks on the sync ring ahead of the
        # first group store (which is only issued ~30us in).
        for sc in range(1, NSC):
            nc.sync.dma_start(
                selib_i[:, sc * SEL_CH : (sc + 1) * SEL_CH, :], sel_i[sc]
            )
            nc.sync.dma_start(
                selib_j[:, sc * SEL_CH : (sc + 1) * SEL_CH, :], sel_j[sc]
            )
        nc.sync.dma_start(selib_t[:], sel_t[:])

        # group-0 selector casts on the DVE (1.2us each, finished well
        # before the stage-A copies are ready, so no FIFO collision); all
        # later groups + the tail on GPSIMD (~3.6us each, serial, but the
        # deadline for group g is ~1.5us * 8g after the stream starts).
        nc.vector.tensor_copy(seli_mm[:, 0:SEL_CH, :], selib_i[:, 0:SEL_CH, :])
        nc.vector.tensor_copy(selj_mm[:, 0:SEL_CH, :], selib_j[:, 0:SEL_CH, :])
        for sc in range(1, NSC):
            sl = slice(sc * SEL_CH, (sc + 1) * SEL_CH)
            nc.gpsimd.tensor_copy(seli_mm[:, sl, :], selib_i[:, sl, :])
            nc.gpsimd.tensor_copy(selj_mm[:, sl, :], selib_j[:, sl, :])
            if sc == 3:
                # tail selector cast before chunk 4: the tail tile is
                # processed mid-stream at group 3 (~tile 28's slot).
                nc.gpsimd.tensor_copy(
                    seli_mm[:, NFT : NFT + 1, :], selib_t[:, 0:1, :]
                )
                nc.gpsimd.tensor_copy(
                    selj_mm[:, NFT : NFT + 1, :], selib_t[:, 1:2, :]
                )

        # PE warmup: ~1.5us of junk matmuls so the HAM clock-gate opens
        # before the real matmuls arrive (cold PE runs at half rate).
        warm_ps = psum.tile([128, 2048], F32, name="warm_ps", tag="ps")
        for k in range(28):
            nc.tensor.matmul(
                warm_ps[:, 0:128], lhsT=warm_w[:], rhs=warm_w[:], start=True, stop=True
            )

        for rep in range(repeat):
            # ---- stage A: projections for both batches --------------------
            # Both batches' PSUM tiles live at once (8 banks) and the chunk
            # matmuls interleave across batches, so the accumulation chases
            # the per-chunk W loads down the rings.
            pps = [
                psum.tile([128, 2048], F32, name=f"pp_{rep}_{bb}", tag="ps")
                for bb in range(BPC)
            ]
            # W1 (proj_i) matmuls lead within each chunk so the proj_i
            # accumulation chain finishes first; its PSUM->SBUF copies then
            # start while the W2/bias matmuls are still on the PE.
            for c in range(NCH - 1):
                first = c == 0
                for bb in range(BPC):
                    pp = pps[bb]
                    hT = hid_sb[:, bb, c, :]
                    nc.tensor.matmul(
                        pp[:, 0:512], lhsT=hT, rhs=w_sb[:, c, 0, 0:512],
                        start=first, stop=False,
                    )
                    nc.tensor.matmul(
                        pp[:, 512:768], lhsT=hT, rhs=w_sb[:, c, 0, 512:768],
                        start=first, stop=False,
                    )
                for bb in range(BPC):
                    pp = pps[bb]
                    hT = hid_sb[:, bb, c, :]
                    nc.tensor.matmul(
                        pp[:, 1024:1536], lhsT=hT, rhs=w_sb[:, c, 1, 0:512],
                        start=first, stop=False,
                    )
                    nc.tensor.matmul(
                        pp[:, 1536:1792], lhsT=hT, rhs=w_sb[:, c, 1, 512:768],
                        start=first, stop=False,
                    )
            # last chunk fully PER BATCH: batch 0's W1 stop -> pi0 copy,
            # then its W2+bias stop -> pj0 copy, BEFORE batch 1's last
            # chunk. pp_0's PSUM buffer (which pair-tile 0 reuses) frees
            # as early as possible, so the tanh stream starts sooner.
            c = NCH - 1
            pis, pjs = [], []
            for bb in range(BPC):
                pp = pps[bb]
                hT = hid_sb[:, bb, c, :]
                nc.tensor.matmul(
                    pp[:, 0:512], lhsT=hT, rhs=w_sb[:, c, 0, 0:512],
                    start=False, stop=True,
                )
                nc.tensor.matmul(
                    pp[:, 512:768], lhsT=hT, rhs=w_sb[:, c, 0, 512:768],
                    start=False, stop=True,
                )
                pi = acts.tile([S, H], F16, name=f"pi_{rep}_{bb}")
                nc.vector.tensor_copy(pi[:], pp[:, 0:768])
                pis.append(pi)
                nc.tensor.matmul(
                    pp[:, 1024:1536], lhsT=hT, rhs=w_sb[:, c, 1, 0:512],
                    start=False, stop=False,
                )
                nc.tensor.matmul(
                    pp[:, 1536:1792], lhsT=hT, rhs=w_sb[:, c, 1, 512:768],
                    start=False, stop=False,
                )
                # bias folded in as a K=1 matmul of ones.T @ b
                nc.tensor.matmul(
                    pp[:, 1024:1536], lhsT=ones_sb[:], rhs=bias_sb[:, 0:512],
                    start=False, stop=True,
                )
                nc.tensor.matmul(
                    pp[:, 1536:1792], lhsT=ones_sb[:], rhs=bias_sb[:, 512:768],
                    start=False, stop=True,
                )
                pj = acts.tile([S, H], F16, name=f"pj_{rep}_{bb}")
                nc.scalar.copy(pj[:], pp[:, 1024:1792])
                pjs.append(pj)

            # ---- stage B: pair tiles, both batches per selector load ------
            for g in range(NGRP):
                t0 = g * GRP
                og2 = outs.tile(
                    [128, BPC, GRP, H], F16, name=f"og_{rep}_{g}", tag="og"
                )
                for tt in range(GRP):
                    t = t0 + tt
                    pq = psum.tile(
                        [128, 2048], F32, name=f"pq_{rep}_{t}", tag="ps"
                    )
                    for sel, prs, st in (
                        (seli_mm, pis, True),
                        (selj_mm, pjs, False),
                    ):
                        nc.tensor.matmul(
                            pq[:, 0:512], lhsT=sel[:, t, :],
                            rhs=prs[0][:, 0:512], start=st, stop=not st,
                        )
                        nc.tensor.matmul(
                            pq[:, 512:768], lhsT=sel[:, t, :],
                            rhs=prs[0][:, 512:768], start=st, stop=not st,
                        )
                        nc.tensor.matmul(
                            pq[:, 1024:1536], lhsT=sel[:, t, :],
                            rhs=prs[1][:, 0:512], start=st, stop=not st,
                        )
                        nc.tensor.matmul(
                            pq[:, 1536:1792], lhsT=sel[:, t, :],
                            rhs=prs[1][:, 512:768], start=st, stop=not st,
                        )
                    nc.scalar.activation(
                        og2[:, :, tt, :],
                        pq.rearrange("p (t x) -> p t x", t=2)[:, :, 0:768],
                        TANH,
                    )
                    r0 = t0 * S
                    dsts = [
                        out[bb, r0 : r0 + GRP * S, :].rearrange(
                            "(p g) h -> p g h", p=128
                        )
                        for bb in range(BPC)
                    ]
                    if g == NGRP - 2 and tt in (1, 3, 5, 7):
                        # group 6: 2-tile (0.39MB) pieces as activations land
                        for bb in range(BPC):
                            nc.sync.dma_start(
                                dsts[bb][:, tt - 1 : tt + 1, :],
                                og2[:, bb, tt - 1 : tt + 1, :],
                            )
                    elif g == NGRP - 1 and (tt in (1, 3, 5) or tt >= 6):
                        # group 7: 2-tile pieces, then per-tile at the very
                        # end so the post-compute drain is one tile deep.
                        # The final piece issues from the (now idle) ACT
                        # queue onto the scalar ring and drains in parallel
                        # with the sync ring's tt=6 piece.
                        sl = slice(tt - 1, tt + 1) if tt < 6 else slice(tt, tt + 1)
                        eng = nc.scalar if tt == GRP - 1 else nc.sync
                        for bb in range(BPC):
                            eng.dma_start(dsts[bb][:, sl, :], og2[:, bb, sl, :])
                if g < NGRP - 2:
                    for bb in range(BPC):
                        nc.sync.dma_start(dsts[bb], og2[:, bb, :, :])
                if g == 3:
                    # ---- tail: 64 pairs via selector tile NFT, processed
                    # mid-stream (after the bulk selector load has landed)
                    # so its ACT/store latency hides ----------------------
                    pqt = psum.tile([128, 2048], F32, name=f"pqt_{rep}", tag="ps")
                    for sel, prs, st in (
                        (seli_mm, pis, True),
                        (selj_mm, pjs, False),
                    ):
                        nc.tensor.matmul(
                            pqt[:, 0:512], lhsT=sel[:, NFT, :],
                            rhs=prs[0][:, 0:512], start=st, stop=not st,
                        )
                        nc.tensor.matmul(
                            pqt[:, 512:768], lhsT=sel[:, NFT, :],
                            rhs=prs[0][:, 512:768], start=st, stop=not st,
                        )
                        nc.tensor.matmul(
                            pqt[:, 1024:1536], lhsT=sel[:, NFT, :],
                            rhs=prs[1][:, 0:512], start=st, stop=not st,
                        )
                        nc.tensor.matmul(
                            pqt[:, 1536:1792], lhsT=sel[:, NFT, :],
                            rhs=prs[1][:, 512:768], start=st, stop=not st,
                        )
                    og2t = acts.tile([128, BPC, H], F16, name=f"ogt_{rep}")
                    nc.scalar.activation(
                        og2t[0:TAIL, :, :],
                        pqt.rearrange("p (t x) -> p t x", t=2)[0:TAIL, :, 0:768],
                        TANH,
                    )
                    for bb in range(BPC):
                        nc.sync.dma_start(
                            out[bb, NFT * S : P, :], og2t[0:TAIL, bb, :]
                        )

    nc.compile()
    return nc


_NC_CACHE: dict[int, bass.Bass] = {}
LAST_RESULTS = None  # BassKernelResults of the most recent kernel() call


def _get_nc(repeat: int = 1) -> bass.Bass:
    if repeat not in _NC_CACHE:
        _NC_CACHE[repeat] = _build_nc(repeat)
    return _NC_CACHE[repeat]


_SEL_CACHE = None


def _selectors() -> tuple[np.ndarray, np.ndarray, np.ndarray]:
    """0/1 selector matrices, fp8 (exact), chunk-major [NSC, S, SEL_CH, S]
    (chunk, token k, tile-within-chunk, column m) so each chunk load has
    2KB-contiguous per-partition lines. Tile t<64 column m selects pair
    1024*(t//8) + 8*m + (t%8); the tail selectors (pair 8192+m in column
    m<64) ride sel_t[S, 2, S] = (k, {i,j}, m)."""
    global _SEL_CACHE
    if _SEL_CACHE is not None:
        return _SEL_CACHE
    import ml_dtypes

    ii, jj = np.triu_indices(S)
    sel_i = np.zeros((NSC, S, SEL_CH, S), dtype=np.float32)
    sel_j = np.zeros((NSC, S, SEL_CH, S), dtype=np.float32)
    m = np.arange(S)
    for t in range(NFT):
        pr = 1024 * (t // 8) + 8 * m + (t % 8)
        sel_i[t // SEL_CH, ii[pr], t % SEL_CH, m] = 1.0
        sel_j[t // SEL_CH, jj[pr], t % SEL_CH, m] = 1.0
    sel_t = np.zeros((S, 2, S), dtype=np.float32)
    mt = np.arange(TAIL)
    pr = NFT * S + mt
    sel_t[ii[pr], 0, mt] = 1.0
    sel_t[jj[pr], 1, mt] = 1.0
    _SEL_CACHE = (
        sel_i.astype(ml_dtypes.float8_e4m3),
        sel_j.astype(ml_dtypes.float8_e4m3),
        sel_t.astype(ml_dtypes.float8_e4m3),
    )
    return _SEL_CACHE


def kernel(hidden: np.ndarray, W: np.ndarray, b: np.ndarray) -> np.ndarray:
    hidden = np.asarray(hidden, dtype=np.float32)
    W = np.asarray(W, dtype=np.float32)
    b = np.asarray(b, dtype=np.float32)

    sel_i, sel_j, sel_t = _selectors()
    # hidden packed per core to [128(k), BPC, NCH, S]: per-partition lines
    # are BPC*NCH*S*2 = 3KB contiguous, loaded in one dma_start.
    # hidden[b, s, d] with d = c*128 + k  ->  hidt[k, b, c, s]
    hidt = np.ascontiguousarray(
        hidden.transpose(2, 0, 1)  # [D, B, S]
        .reshape(D // 128, 128, B, S)
        .transpose(1, 2, 0, 3)  # [128(k), B, NCH, S]
        .astype(np.float16)
    )
    # W packed to [NCH, 128(k), 2(half), H]: one 3KB-line load per chunk.
    # W[d, h] with d = half*768 + c*128 + k -> w[c, k, half, h]
    w16 = np.ascontiguousarray(
        W.reshape(2, D // 128, 128, H).transpose(1, 2, 0, 3).astype(np.float16)
    )
    b16 = b.astype(np.float16).reshape(1, H)

    nc = _get_nc()
    in_maps = []
    for c in range(NCORES):
        in_maps.append(
            {
                "hidt": np.ascontiguousarray(hidt[:, c * BPC : (c + 1) * BPC]),
                "w": w16,
                "bias": b16,
                "sel_i": sel_i,
                "sel_j": sel_j,
                "sel_t": sel_t,
            }
        )
    res = run_bass_kernel_spmd(nc, in_maps, list(range(NCORES)))
    global LAST_RESULTS
    LAST_RESULTS = res
    out = np.concatenate([res.results[c]["out"] for c in range(NCORES)], axis=0)
    return out.astype(np.float32)
